# revision 92
# baseline (speedup 1.0000x reference)
"""NT-Xent contrastive loss kernel for 8 Trainium2 NeuronCores.

Reference computation (N=8192, D=512, tau=0.5):
    zl = l2norm_rows(left); zr = l2norm_rows(right)
    refl    = exp(zl @ zl.T / tau)
    between = exp(zl @ zr.T / tau)
    denom   = refl.sum(1) + between.sum(1) - diag(refl)
    loss    = -log(diag(between) / denom)

Fused per-row form used here (diag(refl) == exp(1/tau) == e^2 exactly since
rows of zl are unit-norm):
    loss[m] = log( S_l[m] + S_r[m] - e^2 ) - 2 * (zl_m . zr_m)
with S_x[m] = sum_n exp(2 * zl_m . zx_n).  The NxN similarity matrices are
never materialized.

Device strategy (v6):
  * Host prep: row-normalize in f32, quantize to fp8-e4m3, lay the transposed
    tensors out K-major as [128(ki), 4(ksub), n] so TensorE runs fp8
    DoubleRow matmuls (contraction 256/instruction, 2 MACs/cell/cycle).
  * Row-parallel: core c owns rows [c*1024, (c+1)*1024).  between_sim has no
    structure, so each core sweeps all 8192 zr columns.  refl_sim is
    SYMMETRIC: with each core's zl copy column-rotated so its own block sits
    first, core c computes only blocks at circulant distance d=0..4 (5120 of
    8192 columns).  Rows' sums over the skipped d=5..7 blocks equal COLUMN
    sums of the d=1..3 tiles computed by cores c-1..c-3: each core folds
    those exp tiles over its m-tiles (Pool/VectorE adds), reduces partitions
    with a ones-matmul, and the [3, 1024] contribution vectors are exchanged
    with a single 12KB AllGather; a per-core 0/1 selection matmul re-routes
    the gathered rows (static SPMD program, per-core routing data).
  * Each [128, 2048] PSUM group (4 banks) is consumed by one 2048-wide
    ScalarE Exp into bf16 scratch; row sums use the activation accumulator
    or a VectorE reduce, balancing engine load.  PSUM ping-pongs 2 groups so
    PE fill and ScalarE drain overlap.
  * The between-diagonal (zl_m . zr_m) comes from one [128,128] PE tile per
    m-tile, diagonal extracted by multiply-with-identity + row reduce.
  * Epilogue: one Ln activation (bias = -e^2) + one fused (-2*bd + log)
    VectorE op, then a single DMA of the [128, 8] losses.
"""

import numpy as np
import ml_dtypes
from contextlib import ExitStack

import concourse.bass as bass
import concourse.tile as tile
from concourse import bacc, mybir
from concourse.bass import ds, ts
from concourse.bass_utils import run_bass_kernel_spmd
from concourse.masks import make_identity

P = 128          # partitions
D = 512          # feature dim
N = 8192         # rows
NCORES = 8
BLK = N // NCORES          # 1024 rows per core
MT = BLK // P              # 8 m-tiles per core
KSUB = D // P              # 4 k-subtiles of 128
NGCOL = 2048               # columns per psum group (4 banks)
RCOLS = 5 * BLK            # refl sweep: blocks at distance 0..4
CSCOLS = 3 * BLK           # colsum region: distance 1..3 blocks
E2 = float(np.exp(2.0))    # exp(sim/tau) on the refl diagonal

F32 = mybir.dt.float32
BF16 = mybir.dt.bfloat16
FP8 = mybir.dt.float8e4
AF = mybir.ActivationFunctionType
OP = mybir.AluOpType
DR = mybir.MatmulPerfMode.DoubleRow

# refl groups per m-tile: (col offset, width)
REFL_GROUPS = [(0, 2048), (2048, 2048), (4096, 1024)]

_CACHE = {}


def _body(ctx, tc, znl_d, znr_d, sel_d, loss_out):
    nc = tc.nc

    const_pool = ctx.enter_context(tc.tile_pool(name="const", bufs=1))
    persist = ctx.enter_context(tc.tile_pool(name="persist", bufs=1))
    small = ctx.enter_context(tc.tile_pool(name="small", bufs=2))
    ex_pool = ctx.enter_context(tc.tile_pool(name="exps", bufs=4))
    psum_mm = ctx.enter_context(tc.tile_pool(name="psmm", bufs=2, space="PSUM"))
    dram = ctx.enter_context(tc.tile_pool(name="dram", bufs=1, space="DRAM"))

    # constants (Pool-queue order: earliest-needed first)
    neg_e2 = const_pool.tile([P, 1], F32, tag="neg_e2")
    nc.gpsimd.memset(neg_e2[:], -E2)
    ones_col = const_pool.tile([P, 1], BF16, tag="ones_col")
    nc.gpsimd.memset(ones_col[:], 1.0)
    ident = const_pool.tile([P, P], BF16, tag="ident")
    make_identity(nc, ident[:])

    # persistent tensors
    znl = persist.tile([P, KSUB, RCOLS], FP8, tag="znl")
    znr = persist.tile([P, KSUB, N], FP8, tag="znr")
    acc = persist.tile([P, CSCOLS], BF16, tag="acc")     # colsum fold accum
    acc_own = persist.tile([P, BLK], BF16, tag="acc_own")  # own-block triangle
    rs_all = persist.tile([P, MT, 9], F32, tag="rs_all")  # per-group row sums
    s_all = persist.tile([P, MT], F32, tag="s_all")
    dmul = persist.tile([P, BLK], F32, tag="dmul")      # masked between-diag
    exp2bd = persist.tile([P, MT], F32, tag="exp2bd")
    inv2bd = persist.tile([P, MT], F32, tag="inv2bd")
    dv_a = persist.tile([P, NGCOL], BF16, tag="dv_a")   # DVE poly-exp scratch
    dv_b = persist.tile([P, NGCOL], BF16, tag="dv_b")
    loss_sb = persist.tile([P, MT], F32, tag="loss_sb")
    sel_sb = persist.tile([3 * NCORES, 1], F32, tag="sel_sb")
    cs24 = persist.tile([3 * NCORES, BLK], F32, tag="cs24")
    cs_sb = persist.tile([65, NGCOL], F32, tag="cs_sb")
    cs_row = persist.tile([1, BLK], F32, tag="cs_row")
    cs_pt = persist.tile([P, MT], F32, tag="cs_pt")
    cs_own_pt = persist.tile([P, MT], F32, tag="cs_own_pt")
    cs_both = persist.tile([P, MT], F32, tag="cs_both")

    nc.gpsimd.memset(acc[:], 0.0)
    nc.gpsimd.memset(acc_own[:], 0.0)
    cs_all = dram.tile([3 * NCORES, BLK], F32, tag="cs_all")

    # ---- input DMAs (HWDGE FIFO; ordered to pace the refl sweep) ----------
    nc.sync.dma_start(znl[:, :, ds(0, 512)], znl_d[:, :, ds(0, 512)])
    nc.sync.dma_start(znl[:, :, ds(512, 512)], znl_d[:, :, ds(512, 512)])
    nc.sync.dma_start(znl[:, :, ds(1024, 1024)], znl_d[:, :, ds(1024, 1024)])
    nc.sync.dma_start(znl[:, :, ds(2048, 2048)], znl_d[:, :, ds(2048, 2048)])
    nc.sync.dma_start(znl[:, :, ds(4096, 1024)], znl_d[:, :, ds(4096, 1024)])
    for g in range(4):
        nc.sync.dma_start(znr[:, :, ts(g, NGCOL)], znr_d[:, :, ts(g, NGCOL)])
    nc.sync.dma_start(sel_sb[:], sel_d[:, :])

    # warm the Exp activation table off the critical path
    warm = small.tile([P, 1], F32, tag="warm")
    nc.scalar.activation(warm[:], neg_e2[:], AF.Exp)

    def lhsT(t, c):
        # own rows sit at rotated columns [0, 1024)
        return znl[:, ds(2 * c, 2), ts(t, P)]

    def sim_group(ps, zn, t, c0, w, rel0=0):
        # psum/ex position = source col - c0; slices split on 512 (bank)
        # boundaries, starting from rel0 (own-block triangle skip)
        edges = [rel0]
        nxt = (rel0 // 512 + 1) * 512
        while nxt < w:
            edges.append(nxt)
            nxt += 512
        edges.append(w)
        for c in range(2):
            for lo, hi in zip(edges[:-1], edges[1:]):
                nc.tensor.matmul(
                    ps[:, ds(lo, hi - lo)],
                    lhsT(t, c),
                    zn[:, ds(2 * c, 2), ds(c0 + lo, hi - lo)],
                    start=(c == 0), stop=(c == 1), perf_mode=DR,
                )

    def fold_ex(ex, c0, w, t, rel0):
        # accumulate exp tiles of circulant-distance-1..3 blocks into `acc`
        # (block 1 on Pool, blocks 2-3 on VectorE, spreading the load)
        for off in range(0, w, BLK):
            b = (c0 + off) // BLK
            if 1 <= b <= 3:
                dst = acc[:, ds((b - 1) * BLK, BLK)]
                eng = nc.gpsimd if b == 1 else nc.vector
                eng.tensor_tensor(dst, dst, ex[:, ds(off, BLK)], op=OP.add)
        # own-block strict-upper columns feed the local colsum accumulator
        lo = max(c0 + rel0, (t + 1) * P)
        hi = min(c0 + w, BLK)
        if lo < hi:
            nc.gpsimd.tensor_tensor(
                acc_own[:, ds(lo, hi - lo)], acc_own[:, ds(lo, hi - lo)],
                ex[:, ds(lo - c0, hi - lo)], op=OP.add,
            )

    # ---- refl phase: rotated cols [0, 5120), col-sum folds for [1024,4096)
    # the first pair uses narrower leading groups so the pipeline starts as
    # soon as the first 1024-column DMA chunk lands.  Within the own block
    # (d0, symmetric against itself) m-tile t computes only columns >= t*128;
    # the strict lower triangle is recovered from acc_own column sums.
    REFL_G0 = [(0, 512), (512, 512), (1024, 1024), (2048, 2048), (4096, 1024)]
    for tp in range(0, MT, 2):
        groups = REFL_G0 if tp == 0 else REFL_GROUPS
        for gi, (c0, w) in enumerate(groups):
            for t in (tp, tp + 1):
                rel0 = max(c0, t * P if c0 < BLK else c0) - c0
                if rel0 >= w:
                    continue
                ps = psum_mm.tile([P, NGCOL], F32, tag="mm")
                sim_group(ps, znl, t, c0, w, rel0)
                rs_slot = rs_all[:, t, ds(gi, 1)]
                ex = ex_pool.tile([P, NGCOL], BF16, tag="ex")
                if gi == len(groups) - 1:
                    # d4 group: skip the accumulator read; VectorE sums the
                    # bf16 scratch (no PSUM hold — the SBUF tile is cheap to
                    # keep alive, unlike a PSUM rotation slot)
                    nc.scalar.activation(
                        ex[:, ds(rel0, w - rel0)], ps[:, ds(rel0, w - rel0)],
                        AF.Exp, scale=2.0,
                    )
                    nc.vector.tensor_reduce(
                        rs_slot, ex[:, ds(rel0, w - rel0)],
                        axis=mybir.AxisListType.X, op=OP.add,
                    )
                else:
                    nc.scalar.activation(
                        ex[:, ds(rel0, w - rel0)], ps[:, ds(rel0, w - rel0)],
                        AF.Exp, scale=2.0, accum_out=rs_slot,
                    )
                    fold_ex(ex, c0, w, t, rel0)

    def colsum_pre():
        # partition-reduce the folded refl tiles and AllGather the [3, 1024]
        # contribution vectors (the collective completes under the between
        # phase; its consumers are emitted after the between loop so the
        # in-order DVE/DMA queues never stall on it).
        # all six [1,512] column-sum tiles share one PSUM rotation: four on
        # partition row 0, two on partition row 32 (tile_position-legal).
        csp1 = psum_mm.tile([P, NGCOL], F32, tag="mm")
        for ni in range(4):
            nc.tensor.matmul(
                csp1[0:1, ds(ni * 512, 512)], ones_col[:],
                acc[:, ds(ni * 512, 512)], start=True, stop=True,
            )
        for ni in range(2):
            nc.tensor.matmul(
                csp1[32:33, ds(ni * 512, 512)], ones_col[:],
                acc[:, ds(2048 + ni * 512, 512)], start=True, stop=True,
            )
        # own-block triangle colsums, read permuted so the [1, 1024] result
        # is already partition-major (position j = p*8 + t)
        acc_own_r = acc_own[:].rearrange("q (t p) -> q p t", t=MT)
        for ni in range(2):
            nc.tensor.matmul(
                csp1[64:65, ds(ni * 512, 512)], ones_col[:],
                acc_own_r[:, ds(ni * 64, 64), :], start=True, stop=True,
            )
        # one wide copy (intermediate rows are don't-care) releases the PSUM
        # buffer sooner than three row copies would
        nc.vector.tensor_copy(cs_sb[:, :], csp1[0:65, :])
        cs_in = dram.tile([1, CSCOLS], F32, tag="cs_in")
        nc.sync.dma_start(cs_in[:, ds(0, 2048)], cs_sb[0:1, :])
        nc.sync.dma_start(cs_in[:, ds(2048, 1024)], cs_sb[32:33, ds(0, 1024)])
        csd_own = dram.tile([1, BLK], F32, tag="csd_own")
        nc.sync.dma_start(csd_own[:], cs_sb[64:65, ds(0, BLK)])
        nc.sync.dma_start(
            cs_own_pt[:, :], csd_own[:].rearrange("a (p t) -> (a p) t", p=P, t=MT)
        )
        nc.gpsimd.collective_compute(
            "AllGather", OP.bypass,
            replica_groups=[list(range(NCORES))],
            ins=[cs_in.opt()], outs=[cs_all.opt()],
        )

    def colsum_post():
        # select this core's rows via a 0/1 matmul and land them as
        # [128, 8] via a DRAM-rearrange bounce.
        nc.sync.dma_start(cs24[:], cs_all[:, :])
        csp3 = psum_mm.tile([P, NGCOL], F32, tag="mm")
        for ni in range(2):
            nc.tensor.matmul(
                csp3[0:1, ds(ni * 512, 512)], sel_sb[:],
                cs24[:, ds(ni * 512, 512)], start=True, stop=True,
            )
        nc.vector.tensor_copy(cs_row[:], csp3[0:1, ds(0, BLK)])
        csd = dram.tile([1, BLK], F32, tag="csd")
        nc.sync.dma_start(csd[:], cs_row[:])
        # position j = p*8 + t (host permuted foreign block columns so the
        # received vector lands partition-major)
        nc.sync.dma_start(
            cs_pt[:, :], csd[:].rearrange("a (p t) -> (a p) t", p=P, t=MT)
        )

    # ---- loss epilogue, split so most rows finish early -------------------
    s_tot = persist.tile([P, MT], F32, tag="s_tot")
    logd = persist.tile([P, MT], F32, tag="logd")

    def epilogue(t0, nt):
        # loss = ln( (S_l + S_r - e^2) / exp(2*zl_m.zr_m) )
        sl = ds(t0, nt)
        nc.vector.tensor_tensor(s_tot[:, sl], s_all[:, sl], cs_both[:, sl],
                                op=OP.add)
        nc.vector.scalar_tensor_tensor(
            out=logd[:, sl], in0=s_tot[:, sl], scalar=-E2,
            in1=inv2bd[:, sl], op0=OP.add, op1=OP.mult,
        )
        nc.scalar.activation(loss_sb[:, sl], logd[:, sl], AF.Ln)
        nc.sync.dma_start(loss_out[:, sl], loss_sb[:, sl])

    # ---- between phase: all 8192 zr columns; colsum chain overlaps -------
    for tp in range(0, MT, 2):
        for g in range(4):
            for t in (tp, tp + 1):
                ps = psum_mm.tile([P, NGCOL], F32, tag="mm")
                sim_group(ps, znr, t, g * NGCOL, NGCOL)
                rs_slot = rs_all[:, t, ds((5 if tp == 0 else 3) + g, 1)]
                if tp != 0 and t == tp and g == 1:
                    # steal one group per pair for a VectorE polynomial exp:
                    # exp(2s) ~= (1 + (s/8)(1 + s/16))^16 — bf16 squarings run
                    # at the DVE 2x rate, freeing ScalarE stream time
                    nc.vector.tensor_scalar(
                        out=dv_a[:], in0=ps[:], scalar1=0.125, scalar2=0.0,
                        op0=OP.mult, op1=OP.add,
                    )
                    nc.vector.tensor_scalar(
                        out=dv_b[:], in0=dv_a[:], scalar1=0.5, scalar2=1.0,
                        op0=OP.mult, op1=OP.add,
                    )
                    nc.vector.tensor_tensor(dv_a[:], dv_b[:], dv_a[:], op=OP.mult)
                    nc.vector.tensor_scalar(
                        out=dv_b[:], in0=dv_a[:], scalar1=1.0, scalar2=0.0,
                        op0=OP.add, op1=OP.add,
                    )
                    for _ in range(2):
                        nc.vector.tensor_tensor(dv_a[:], dv_b[:], dv_b[:], op=OP.mult)
                        nc.vector.tensor_tensor(dv_b[:], dv_a[:], dv_a[:], op=OP.mult)
                    nc.vector.tensor_reduce(
                        rs_slot, dv_b[:], axis=mybir.AxisListType.X, op=OP.add
                    )
                elif g == 0:
                    # g0 holds this m-tile's between-diagonal (znr is rotated
                    # so own rows sit at columns [0, 1024)); mask it out of
                    # the exp scratch before the buffer rotates away
                    ex = ex_pool.tile([P, NGCOL], BF16, tag="ex")
                    nc.scalar.activation(
                        ex[:], ps[:], AF.Exp, scale=2.0, accum_out=rs_slot
                    )
                    nc.vector.tensor_tensor(
                        dmul[:, ts(t, P)], ex[:, ts(t, P)], ident[:],
                        op=OP.mult,
                    )
                elif tp == 0 and g >= 2:
                    ex = ex_pool.tile([P, NGCOL], BF16, tag="ex")
                    nc.scalar.activation(ex[:], ps[:], AF.Exp, scale=2.0)
                    nc.vector.tensor_reduce(
                        rs_slot, ex[:], axis=mybir.AxisListType.X, op=OP.add
                    )
                else:
                    # accum-only group: nobody reads the exp values, so write
                    # them back in place (PSUM access is cheaper than SBUF)
                    nc.scalar.activation(
                        ps[:], ps[:], AF.Exp, scale=2.0, accum_out=rs_slot
                    )
            if tp == 0 and g == 1:
                colsum_pre()
            if tp == MT - 2 and g == 1:
                colsum_post()
                nc.vector.tensor_tensor(
                    cs_both[:], cs_pt[:], cs_own_pt[:], op=OP.add
                )
        nslot = 9 if tp == 0 else 7
        for t in (tp, tp + 1):
            nc.vector.tensor_reduce(
                s_all[:, ts(t, 1)], rs_all[:, t, ds(0, nslot)],
                axis=mybir.AxisListType.X, op=OP.add,
            )
    nc.vector.tensor_reduce(
        exp2bd[:, :], dmul[:].rearrange("p (t q) -> p t q", t=MT),
        axis=mybir.AxisListType.X, op=OP.add,
    )
    nc.vector.reciprocal(inv2bd[:], exp2bd[:])
    epilogue(0, MT)


def _build():
    nc = bacc.Bacc("TRN2", target_bir_lowering=False, debug=False, num_devices=NCORES)
    znl_d = nc.dram_tensor("znl", [P, KSUB, RCOLS], FP8, kind="ExternalInput").ap()
    znr_d = nc.dram_tensor("znr", [P, KSUB, N], FP8, kind="ExternalInput").ap()
    sel_d = nc.dram_tensor("sel", [3 * NCORES, 1], F32, kind="ExternalInput").ap()
    loss = nc.dram_tensor("loss", [P, MT], F32, kind="ExternalOutput").ap()
    with tile.TileContext(nc) as tc, ExitStack() as ctx:
        _body(ctx, tc, znl_d, znr_d, sel_d, loss)
    nc.compile()
    return nc


def _get_nc():
    if "nc" not in _CACHE:
        _CACHE["nc"] = _build()
    return _CACHE["nc"]


def _norm_q8_T(x):
    """Row-normalize [N, D] f32, quantize fp8-e4m3, return K-major
    [128(ki), 4(ksub), N] for DoubleRow matmuls."""
    x = np.asarray(x, dtype=np.float32)
    n = np.sqrt(np.sum(x * x, axis=1, keepdims=True))
    z = x / np.maximum(n, 1e-12)
    q = z.astype(ml_dtypes.float8_e4m3)
    zT = np.ascontiguousarray(q.T)                      # [512, N]
    arr = zT.reshape(KSUB, P, zT.shape[1]).transpose(1, 0, 2)
    return np.ascontiguousarray(arr)


# in-block permutation so received colsum vectors land partition-major:
# position j holds row (j%8)*128 + j//8
_PERM = (np.arange(BLK) % MT) * P + np.arange(BLK) // MT


def _in_maps(left, right):
    znl_g = _norm_q8_T(left)    # [128, 4, 8192]
    znr_g = _norm_q8_T(right)
    maps = []
    for c in range(NCORES):
        rot = np.roll(znl_g, -c * BLK, axis=2)[:, :, :RCOLS].copy()
        for b in (1, 2, 3):  # permute foreign colsum blocks
            blk = rot[:, :, b * BLK:(b + 1) * BLK]
            rot[:, :, b * BLK:(b + 1) * BLK] = blk[:, :, _PERM]
        sel = np.zeros((3 * NCORES, 1), dtype=np.float32)
        for d in (1, 2, 3):
            s = (c - d) % NCORES
            sel[s * 3 + (d - 1), 0] = 1.0
        maps.append({
            "znl": np.ascontiguousarray(rot),
            # znr rotated too: own rows at columns [0, 1024) put the
            # between-diagonal inside each m-tile's first between group
            "znr": np.ascontiguousarray(np.roll(znr_g, -c * BLK, axis=2)),
            "sel": sel,
        })
    return maps


def _gather(results):
    # loss dram tile is [128 partitions, 8 m-tiles]; row m = t*128 + p
    parts = [np.asarray(r["loss"]).T.reshape(-1) for r in results]
    return np.concatenate(parts).astype(np.float32)


def run_traced(left, right):
    """Run with NTFF profiling; returns (loss, exec_time_ns)."""
    res = run_bass_kernel_spmd(
        _get_nc(), _in_maps(left, right), list(range(NCORES)), trace=True
    )
    return _gather(res.results), res.exec_time_ns


def kernel(left, right):
    res = run_bass_kernel_spmd(
        _get_nc(), _in_maps(left, right), list(range(NCORES))
    )
    return _gather(res.results)


# revision 95
# speedup vs baseline: 1.0007x; 1.0007x over previous
"""NT-Xent contrastive loss kernel for 8 Trainium2 NeuronCores.

Reference computation (N=8192, D=512, tau=0.5):
    zl = l2norm_rows(left); zr = l2norm_rows(right)
    refl    = exp(zl @ zl.T / tau)
    between = exp(zl @ zr.T / tau)
    denom   = refl.sum(1) + between.sum(1) - diag(refl)
    loss    = -log(diag(between) / denom)

Fused per-row form used here (diag(refl) == exp(1/tau) == e^2 exactly since
rows of zl are unit-norm):
    loss[m] = log( S_l[m] + S_r[m] - e^2 ) - 2 * (zl_m . zr_m)
with S_x[m] = sum_n exp(2 * zl_m . zx_n).  The NxN similarity matrices are
never materialized.

Device strategy (v6):
  * Host prep: row-normalize in f32, quantize to fp8-e4m3, lay the transposed
    tensors out K-major as [128(ki), 4(ksub), n] so TensorE runs fp8
    DoubleRow matmuls (contraction 256/instruction, 2 MACs/cell/cycle).
  * Row-parallel: core c owns rows [c*1024, (c+1)*1024).  between_sim has no
    structure, so each core sweeps all 8192 zr columns.  refl_sim is
    SYMMETRIC: with each core's zl copy column-rotated so its own block sits
    first, core c computes only blocks at circulant distance d=0..4 (5120 of
    8192 columns).  Rows' sums over the skipped d=5..7 blocks equal COLUMN
    sums of the d=1..3 tiles computed by cores c-1..c-3: each core folds
    those exp tiles over its m-tiles (Pool/VectorE adds), reduces partitions
    with a ones-matmul, and the [3, 1024] contribution vectors are exchanged
    with a single 12KB AllGather; a per-core 0/1 selection matmul re-routes
    the gathered rows (static SPMD program, per-core routing data).
  * Each [128, 2048] PSUM group (4 banks) is consumed by one 2048-wide
    ScalarE Exp into bf16 scratch; row sums use the activation accumulator
    or a VectorE reduce, balancing engine load.  PSUM ping-pongs 2 groups so
    PE fill and ScalarE drain overlap.
  * The between-diagonal (zl_m . zr_m) comes from one [128,128] PE tile per
    m-tile, diagonal extracted by multiply-with-identity + row reduce.
  * Epilogue: one Ln activation (bias = -e^2) + one fused (-2*bd + log)
    VectorE op, then a single DMA of the [128, 8] losses.
"""

import numpy as np
import ml_dtypes
from contextlib import ExitStack

import concourse.bass as bass
import concourse.tile as tile
from concourse import bacc, mybir
from concourse.bass import ds, ts
from concourse.bass_utils import run_bass_kernel_spmd
from concourse.masks import make_identity

P = 128          # partitions
D = 512          # feature dim
N = 8192         # rows
NCORES = 8
BLK = N // NCORES          # 1024 rows per core
MT = BLK // P              # 8 m-tiles per core
KSUB = D // P              # 4 k-subtiles of 128
NGCOL = 2048               # columns per psum group (4 banks)
RCOLS = 5 * BLK            # refl sweep: blocks at distance 0..4
CSCOLS = 3 * BLK           # colsum region: distance 1..3 blocks
E2 = float(np.exp(2.0))    # exp(sim/tau) on the refl diagonal

F32 = mybir.dt.float32
BF16 = mybir.dt.bfloat16
FP8 = mybir.dt.float8e4
AF = mybir.ActivationFunctionType
OP = mybir.AluOpType
DR = mybir.MatmulPerfMode.DoubleRow

# refl groups per m-tile: (col offset, width)
REFL_GROUPS = [(0, 2048), (2048, 2048), (4096, 1024)]

_CACHE = {}


def _body(ctx, tc, znl_d, znr_d, sel_d, loss_out):
    nc = tc.nc

    const_pool = ctx.enter_context(tc.tile_pool(name="const", bufs=1))
    persist = ctx.enter_context(tc.tile_pool(name="persist", bufs=1))
    small = ctx.enter_context(tc.tile_pool(name="small", bufs=2))
    ex_pool = ctx.enter_context(tc.tile_pool(name="exps", bufs=5))
    psum_mm = ctx.enter_context(tc.tile_pool(name="psmm", bufs=2, space="PSUM"))
    dram = ctx.enter_context(tc.tile_pool(name="dram", bufs=1, space="DRAM"))

    # constants (Pool-queue order: earliest-needed first)
    neg_e2 = const_pool.tile([P, 1], F32, tag="neg_e2")
    nc.gpsimd.memset(neg_e2[:], -E2)
    ones_col = const_pool.tile([P, 1], BF16, tag="ones_col")
    nc.gpsimd.memset(ones_col[:], 1.0)
    ident = const_pool.tile([P, P], BF16, tag="ident")
    make_identity(nc, ident[:])

    # persistent tensors
    znl = persist.tile([P, KSUB, RCOLS], FP8, tag="znl")
    znr = persist.tile([P, KSUB, N], FP8, tag="znr")
    acc = persist.tile([P, CSCOLS], BF16, tag="acc")     # colsum fold accum
    acc_own = persist.tile([P, BLK], BF16, tag="acc_own")  # own-block triangle
    rs_all = persist.tile([P, MT, 9], F32, tag="rs_all")  # per-group row sums
    s_all = persist.tile([P, MT], F32, tag="s_all")
    dmul = persist.tile([P, BLK], F32, tag="dmul")      # masked between-diag
    exp2bd = persist.tile([P, MT], F32, tag="exp2bd")
    inv2bd = persist.tile([P, MT], F32, tag="inv2bd")
    dv_a = persist.tile([P, NGCOL], BF16, tag="dv_a")   # DVE poly-exp scratch
    dv_b = persist.tile([P, NGCOL], BF16, tag="dv_b")
    loss_sb = persist.tile([P, MT], F32, tag="loss_sb")
    sel_sb = persist.tile([3 * NCORES, 1], F32, tag="sel_sb")
    cs24 = persist.tile([3 * NCORES, BLK], F32, tag="cs24")
    cs_sb = persist.tile([65, NGCOL], F32, tag="cs_sb")
    cs_row = persist.tile([1, BLK], F32, tag="cs_row")
    cs_pt = persist.tile([P, MT], F32, tag="cs_pt")
    cs_own_pt = persist.tile([P, MT], F32, tag="cs_own_pt")
    cs_both = persist.tile([P, MT], F32, tag="cs_both")

    nc.gpsimd.memset(acc[:], 0.0)
    nc.gpsimd.memset(acc_own[:], 0.0)
    cs_all = dram.tile([3 * NCORES, BLK], F32, tag="cs_all")

    # ---- input DMAs (HWDGE FIFO; ordered to pace the refl sweep) ----------
    nc.sync.dma_start(znl[:, :, ds(0, 512)], znl_d[:, :, ds(0, 512)])
    nc.sync.dma_start(znl[:, :, ds(512, 512)], znl_d[:, :, ds(512, 512)])
    nc.sync.dma_start(znl[:, :, ds(1024, 1024)], znl_d[:, :, ds(1024, 1024)])
    nc.sync.dma_start(znl[:, :, ds(2048, 2048)], znl_d[:, :, ds(2048, 2048)])
    nc.sync.dma_start(znl[:, :, ds(4096, 1024)], znl_d[:, :, ds(4096, 1024)])
    for g in range(4):
        nc.sync.dma_start(znr[:, :, ts(g, NGCOL)], znr_d[:, :, ts(g, NGCOL)])
    nc.sync.dma_start(sel_sb[:], sel_d[:, :])

    # warm the Exp activation table off the critical path
    warm = small.tile([P, 1], F32, tag="warm")
    nc.scalar.activation(warm[:], neg_e2[:], AF.Exp)

    def lhsT(t, c):
        # own rows sit at rotated columns [0, 1024)
        return znl[:, ds(2 * c, 2), ts(t, P)]

    def sim_group(ps, zn, t, c0, w, rel0=0):
        # psum/ex position = source col - c0; slices split on 512 (bank)
        # boundaries, starting from rel0 (own-block triangle skip)
        edges = [rel0]
        nxt = (rel0 // 512 + 1) * 512
        while nxt < w:
            edges.append(nxt)
            nxt += 512
        edges.append(w)
        for c in range(2):
            for lo, hi in zip(edges[:-1], edges[1:]):
                nc.tensor.matmul(
                    ps[:, ds(lo, hi - lo)],
                    lhsT(t, c),
                    zn[:, ds(2 * c, 2), ds(c0 + lo, hi - lo)],
                    start=(c == 0), stop=(c == 1), perf_mode=DR,
                )

    def fold_ex(ex, c0, w, t, rel0):
        # accumulate exp tiles of circulant-distance-1..3 blocks into `acc`
        # (block 1 on Pool, blocks 2-3 on VectorE, spreading the load)
        for off in range(0, w, BLK):
            b = (c0 + off) // BLK
            if 1 <= b <= 3:
                dst = acc[:, ds((b - 1) * BLK, BLK)]
                eng = nc.gpsimd if b == 1 else nc.vector
                eng.tensor_tensor(dst, dst, ex[:, ds(off, BLK)], op=OP.add)
        # own-block strict-upper columns feed the local colsum accumulator
        lo = max(c0 + rel0, (t + 1) * P)
        hi = min(c0 + w, BLK)
        if lo < hi:
            nc.gpsimd.tensor_tensor(
                acc_own[:, ds(lo, hi - lo)], acc_own[:, ds(lo, hi - lo)],
                ex[:, ds(lo - c0, hi - lo)], op=OP.add,
            )

    # ---- refl phase: rotated cols [0, 5120), col-sum folds for [1024,4096)
    # the first pair uses narrower leading groups so the pipeline starts as
    # soon as the first 1024-column DMA chunk lands.  Within the own block
    # (d0, symmetric against itself) m-tile t computes only columns >= t*128;
    # the strict lower triangle is recovered from acc_own column sums.
    REFL_G0 = [(0, 512), (512, 512), (1024, 1024), (2048, 2048), (4096, 1024)]
    for tp in range(0, MT, 2):
        groups = REFL_G0 if tp == 0 else REFL_GROUPS
        for gi, (c0, w) in enumerate(groups):
            for t in (tp, tp + 1):
                rel0 = max(c0, t * P if c0 < BLK else c0) - c0
                if rel0 >= w:
                    continue
                ps = psum_mm.tile([P, NGCOL], F32, tag="mm")
                sim_group(ps, znl, t, c0, w, rel0)
                rs_slot = rs_all[:, t, ds(gi, 1)]
                ex = ex_pool.tile([P, NGCOL], BF16, tag="ex")
                if gi == len(groups) - 1:
                    # d4 group: skip the accumulator read; VectorE sums the
                    # bf16 scratch (no PSUM hold — the SBUF tile is cheap to
                    # keep alive, unlike a PSUM rotation slot)
                    nc.scalar.activation(
                        ex[:, ds(rel0, w - rel0)], ps[:, ds(rel0, w - rel0)],
                        AF.Exp, scale=2.0,
                    )
                    nc.vector.tensor_reduce(
                        rs_slot, ex[:, ds(rel0, w - rel0)],
                        axis=mybir.AxisListType.X, op=OP.add,
                    )
                else:
                    nc.scalar.activation(
                        ex[:, ds(rel0, w - rel0)], ps[:, ds(rel0, w - rel0)],
                        AF.Exp, scale=2.0, accum_out=rs_slot,
                    )
                    fold_ex(ex, c0, w, t, rel0)

    def colsum_pre():
        # partition-reduce the folded refl tiles and AllGather the [3, 1024]
        # contribution vectors (the collective completes under the between
        # phase; its consumers are emitted after the between loop so the
        # in-order DVE/DMA queues never stall on it).
        # all six [1,512] column-sum tiles share one PSUM rotation: four on
        # partition row 0, two on partition row 32 (tile_position-legal).
        csp1 = psum_mm.tile([P, NGCOL], F32, tag="mm")
        for ni in range(4):
            nc.tensor.matmul(
                csp1[0:1, ds(ni * 512, 512)], ones_col[:],
                acc[:, ds(ni * 512, 512)], start=True, stop=True,
            )
        for ni in range(2):
            nc.tensor.matmul(
                csp1[32:33, ds(ni * 512, 512)], ones_col[:],
                acc[:, ds(2048 + ni * 512, 512)], start=True, stop=True,
            )
        # own-block triangle colsums, read permuted so the [1, 1024] result
        # is already partition-major (position j = p*8 + t)
        acc_own_r = acc_own[:].rearrange("q (t p) -> q p t", t=MT)
        for ni in range(2):
            nc.tensor.matmul(
                csp1[64:65, ds(ni * 512, 512)], ones_col[:],
                acc_own_r[:, ds(ni * 64, 64), :], start=True, stop=True,
            )
        # one wide copy (intermediate rows are don't-care) releases the PSUM
        # buffer sooner than three row copies would
        nc.vector.tensor_copy(cs_sb[:, :], csp1[0:65, :])
        cs_in = dram.tile([1, CSCOLS], F32, tag="cs_in")
        nc.sync.dma_start(cs_in[:, ds(0, 2048)], cs_sb[0:1, :])
        nc.sync.dma_start(cs_in[:, ds(2048, 1024)], cs_sb[32:33, ds(0, 1024)])
        csd_own = dram.tile([1, BLK], F32, tag="csd_own")
        nc.sync.dma_start(csd_own[:], cs_sb[64:65, ds(0, BLK)])
        nc.sync.dma_start(
            cs_own_pt[:, :], csd_own[:].rearrange("a (p t) -> (a p) t", p=P, t=MT)
        )
        nc.gpsimd.collective_compute(
            "AllGather", OP.bypass,
            replica_groups=[list(range(NCORES))],
            ins=[cs_in.opt()], outs=[cs_all.opt()],
        )

    def colsum_post():
        # select this core's rows via a 0/1 matmul and land them as
        # [128, 8] via a DRAM-rearrange bounce.
        nc.sync.dma_start(cs24[:], cs_all[:, :])
        csp3 = psum_mm.tile([P, NGCOL], F32, tag="mm")
        for ni in range(2):
            nc.tensor.matmul(
                csp3[0:1, ds(ni * 512, 512)], sel_sb[:],
                cs24[:, ds(ni * 512, 512)], start=True, stop=True,
            )
        nc.vector.tensor_copy(cs_row[:], csp3[0:1, ds(0, BLK)])
        csd = dram.tile([1, BLK], F32, tag="csd")
        nc.sync.dma_start(csd[:], cs_row[:])
        # position j = p*8 + t (host permuted foreign block columns so the
        # received vector lands partition-major)
        nc.sync.dma_start(
            cs_pt[:, :], csd[:].rearrange("a (p t) -> (a p) t", p=P, t=MT)
        )

    # ---- loss epilogue, split so most rows finish early -------------------
    s_tot = persist.tile([P, MT], F32, tag="s_tot")
    logd = persist.tile([P, MT], F32, tag="logd")

    def epilogue(t0, nt):
        # loss = ln( (S_l + S_r - e^2) / exp(2*zl_m.zr_m) )
        sl = ds(t0, nt)
        nc.vector.tensor_tensor(s_tot[:, sl], s_all[:, sl], cs_both[:, sl],
                                op=OP.add)
        nc.vector.scalar_tensor_tensor(
            out=logd[:, sl], in0=s_tot[:, sl], scalar=-E2,
            in1=inv2bd[:, sl], op0=OP.add, op1=OP.mult,
        )
        nc.scalar.activation(loss_sb[:, sl], logd[:, sl], AF.Ln)
        nc.sync.dma_start(loss_out[:, sl], loss_sb[:, sl])

    # ---- between phase: all 8192 zr columns; colsum chain overlaps -------
    for tp in range(0, MT, 2):
        for g in range(4):
            for t in (tp, tp + 1):
                ps = psum_mm.tile([P, NGCOL], F32, tag="mm")
                sim_group(ps, znr, t, g * NGCOL, NGCOL)
                rs_slot = rs_all[:, t, ds((5 if tp == 0 else 3) + g, 1)]
                if tp != 0 and t == tp and g == 1:
                    # steal one group per pair for a VectorE polynomial exp:
                    # exp(2s) ~= (1 + (s/8)(1 + s/16))^16 — bf16 squarings run
                    # at the DVE 2x rate, freeing ScalarE stream time
                    nc.vector.tensor_scalar(
                        out=dv_a[:], in0=ps[:], scalar1=0.125, scalar2=0.0,
                        op0=OP.mult, op1=OP.add,
                    )
                    nc.vector.tensor_scalar(
                        out=dv_b[:], in0=dv_a[:], scalar1=0.5, scalar2=1.0,
                        op0=OP.mult, op1=OP.add,
                    )
                    nc.vector.tensor_tensor(dv_a[:], dv_b[:], dv_a[:], op=OP.mult)
                    nc.vector.tensor_scalar(
                        out=dv_b[:], in0=dv_a[:], scalar1=1.0, scalar2=0.0,
                        op0=OP.add, op1=OP.add,
                    )
                    for _ in range(2):
                        nc.vector.tensor_tensor(dv_a[:], dv_b[:], dv_b[:], op=OP.mult)
                        nc.vector.tensor_tensor(dv_b[:], dv_a[:], dv_a[:], op=OP.mult)
                    nc.vector.tensor_reduce(
                        rs_slot, dv_b[:], axis=mybir.AxisListType.X, op=OP.add
                    )
                elif g == 0:
                    # g0 holds this m-tile's between-diagonal (znr is rotated
                    # so own rows sit at columns [0, 1024)); mask it out of
                    # the exp scratch before the buffer rotates away
                    ex = ex_pool.tile([P, NGCOL], BF16, tag="ex")
                    nc.scalar.activation(
                        ex[:], ps[:], AF.Exp, scale=2.0, accum_out=rs_slot
                    )
                    nc.vector.tensor_tensor(
                        dmul[:, ts(t, P)], ex[:, ts(t, P)], ident[:],
                        op=OP.mult,
                    )
                elif tp == 0 and g >= 2:
                    ex = ex_pool.tile([P, NGCOL], BF16, tag="ex")
                    nc.scalar.activation(ex[:], ps[:], AF.Exp, scale=2.0)
                    nc.vector.tensor_reduce(
                        rs_slot, ex[:], axis=mybir.AxisListType.X, op=OP.add
                    )
                else:
                    # accum-only group: nobody reads the exp values, so write
                    # them back in place (PSUM access is cheaper than SBUF)
                    nc.scalar.activation(
                        ps[:], ps[:], AF.Exp, scale=2.0, accum_out=rs_slot
                    )
            if tp == 0 and g == 1:
                colsum_pre()
            if tp == MT - 2 and g == 1:
                colsum_post()
                nc.vector.tensor_tensor(
                    cs_both[:], cs_pt[:], cs_own_pt[:], op=OP.add
                )
        nslot = 9 if tp == 0 else 7
        for t in (tp, tp + 1):
            nc.vector.tensor_reduce(
                s_all[:, ts(t, 1)], rs_all[:, t, ds(0, nslot)],
                axis=mybir.AxisListType.X, op=OP.add,
            )
    nc.vector.tensor_reduce(
        exp2bd[:, :], dmul[:].rearrange("p (t q) -> p t q", t=MT),
        axis=mybir.AxisListType.X, op=OP.add,
    )
    nc.vector.reciprocal(inv2bd[:], exp2bd[:])
    epilogue(0, MT)


def _build():
    nc = bacc.Bacc("TRN2", target_bir_lowering=False, debug=False, num_devices=NCORES)
    znl_d = nc.dram_tensor("znl", [P, KSUB, RCOLS], FP8, kind="ExternalInput").ap()
    znr_d = nc.dram_tensor("znr", [P, KSUB, N], FP8, kind="ExternalInput").ap()
    sel_d = nc.dram_tensor("sel", [3 * NCORES, 1], F32, kind="ExternalInput").ap()
    loss = nc.dram_tensor("loss", [P, MT], F32, kind="ExternalOutput").ap()
    with tile.TileContext(nc) as tc, ExitStack() as ctx:
        _body(ctx, tc, znl_d, znr_d, sel_d, loss)
    nc.compile()
    return nc


def _get_nc():
    if "nc" not in _CACHE:
        _CACHE["nc"] = _build()
    return _CACHE["nc"]


def _norm_q8_T(x):
    """Row-normalize [N, D] f32, quantize fp8-e4m3, return K-major
    [128(ki), 4(ksub), N] for DoubleRow matmuls."""
    x = np.asarray(x, dtype=np.float32)
    n = np.sqrt(np.sum(x * x, axis=1, keepdims=True))
    z = x / np.maximum(n, 1e-12)
    q = z.astype(ml_dtypes.float8_e4m3)
    zT = np.ascontiguousarray(q.T)                      # [512, N]
    arr = zT.reshape(KSUB, P, zT.shape[1]).transpose(1, 0, 2)
    return np.ascontiguousarray(arr)


# in-block permutation so received colsum vectors land partition-major:
# position j holds row (j%8)*128 + j//8
_PERM = (np.arange(BLK) % MT) * P + np.arange(BLK) // MT


def _in_maps(left, right):
    znl_g = _norm_q8_T(left)    # [128, 4, 8192]
    znr_g = _norm_q8_T(right)
    maps = []
    for c in range(NCORES):
        rot = np.roll(znl_g, -c * BLK, axis=2)[:, :, :RCOLS].copy()
        for b in (1, 2, 3):  # permute foreign colsum blocks
            blk = rot[:, :, b * BLK:(b + 1) * BLK]
            rot[:, :, b * BLK:(b + 1) * BLK] = blk[:, :, _PERM]
        sel = np.zeros((3 * NCORES, 1), dtype=np.float32)
        for d in (1, 2, 3):
            s = (c - d) % NCORES
            sel[s * 3 + (d - 1), 0] = 1.0
        maps.append({
            "znl": np.ascontiguousarray(rot),
            # znr rotated too: own rows at columns [0, 1024) put the
            # between-diagonal inside each m-tile's first between group
            "znr": np.ascontiguousarray(np.roll(znr_g, -c * BLK, axis=2)),
            "sel": sel,
        })
    return maps


def _gather(results):
    # loss dram tile is [128 partitions, 8 m-tiles]; row m = t*128 + p
    parts = [np.asarray(r["loss"]).T.reshape(-1) for r in results]
    return np.concatenate(parts).astype(np.float32)


def run_traced(left, right):
    """Run with NTFF profiling; returns (loss, exec_time_ns)."""
    res = run_bass_kernel_spmd(
        _get_nc(), _in_maps(left, right), list(range(NCORES)), trace=True
    )
    return _gather(res.results), res.exec_time_ns


def kernel(left, right):
    res = run_bass_kernel_spmd(
        _get_nc(), _in_maps(left, right), list(range(NCORES))
    )
    return _gather(res.results)
